# revision 1
# baseline (speedup 1.0000x reference)
"""Trainium2 Bass kernel for nn_Graph_to_Featuremaps_savemem.

Reference computation:
    scores[b,p,n] = s_res[b,p] + s_hid[b,n];  attn = softmax_n(scores)
    out[b,c,p]    = relu(sum_n attn[b,p,n] * (x[b,n,:] @ W)[c])

Key simplification: softmax over n is shift-invariant, so the per-pixel
s_res[b,p] term (the only use of res_feature / node_fea_for_res) cancels:
    attn[b,p,n] = softmax_n(s_hid[b,n])   (independent of p)
    out[b,c,p]  = relu(sum_n a[b,n] * nv[b,n,c])  broadcast over all pixels.

So the kernel is a tiny softmax-weighted matmul (per-batch (7,256)x(256,256))
followed by a 151 MB broadcast-write of the (B,C) result over H*W pixels.
Sharding: data-parallel over batch, 2 batches per core across 8 cores; the
small params (node_fea_for_hidden, weight) are replicated.

Hardware constraints shaping the structure:
- PE matmul / tensor-scalar / DMA-trigger instructions have a single
  sync-wait slot, so every PE operand pair must share one producer
  semaphore. All small inputs (w, x, nfh, identity, block-mask, ones) are
  packed host-side into ONE DRAM tensor loaded by ONE DMA; PSUM results are
  funneled through DVE copies.
- The kernel-tail drain also has limited wait slots, so the kernel keeps the
  total semaphore count low: only ACT (which triggers all DMAs), PE, DVE and
  the 8 HW DMA queues are used.
- matmul operands need base partition 0/32/64; x and the transpose identity
  live at rows 32:46 of the packed tile, everything else at base 0.
"""

import numpy as np

import concourse.bass as bass
import concourse.mybir as mybir
import concourse.tile as tile
from concourse.bass_utils import run_bass_kernel_spmd

B, NODES, HID, C, H, W = 16, 7, 256, 256, 96, 96
P = H * W                # 9216 pixels
NCORES = 8
BL = B // NCORES         # 2 local batches per core
BN = BL * NODES          # 14 (b,n) rows
WCHUNK = 9216            # broadcast tile width; P = 1 * WCHUNK
NCHUNK = P // WCHUNK

# Packed input layout: (128, CIN_COLS) float32
COL_W = 0        # cols 0:512, all rows: w[kh*128+k, c] at [k, kh*256+c]
COL_ID = 512     # cols 512:526, rows 32:46: identity(14)
COL_BM = 526     # cols 526:528, rows 0:14: block-diagonal mask (14, 2)
COL_XN = 528     # cols 528:784: row 0 = nfh; rows 32:46 = x[(b n), h]
COL_ONE = 784    # col 784, row 0: 1.0
CIN_COLS = 785
XROW = 32        # base partition for x / identity (must be 0, 32 or 64)

_cache: dict = {}


def _build_nc():
    nc = bass.Bass()
    dt = mybir.dt.float32
    cin_d = nc.declare_dram_parameter("cin", [128, CIN_COLS], dt, isOutput=False)
    out_d = nc.declare_dram_parameter("out", [BL, C, P], dt, isOutput=True)

    with tile.TileContext(nc) as tc:
        with (
            tc.tile_pool(name="sb", bufs=1) as sb,
            tc.tile_pool(name="ps", bufs=1, space=bass.MemorySpace.PSUM) as ps,
        ):
            cin = sb.tile([128, CIN_COLS], dt)
            nc.scalar.dma_start(out=cin[:], in_=cin_d[:])
            x_sl = cin[XROW : XROW + BN, COL_XN : COL_XN + HID]
            ident = cin[XROW : XROW + BN, COL_ID : COL_ID + BN]
            nfh_row = cin[0:1, COL_XN : COL_XN + HID]
            one_cin = cin[0:1, COL_ONE : COL_ONE + 1]

            ones11 = sb.tile([1, 1], dt)
            nc.vector.memset(ones11[:], 1.0)
            sb_w = sb.tile([128, 2 * C], dt)
            nc.vector.tensor_copy(out=sb_w[:], in_=cin[:, 0 : 2 * C])
            blkmask = sb.tile([BN, BL], dt)
            nc.vector.tensor_copy(out=blkmask[:], in_=cin[0:BN, COL_BM : COL_BM + BL])

            # PE-transpose x to (h, bn) layout, one (128, 14) tile per k-half.
            sbT = []
            for kh in range(2):
                p_t = ps.tile([128, BN], dt, tag=f"xT{kh}")
                nc.tensor.transpose(p_t[:], x_sl[:, kh * 128 : (kh + 1) * 128], ident)
                s_t = sb.tile([128, BN], dt, tag=f"sbT{kh}")
                nc.vector.tensor_copy(out=s_t[:], in_=p_t[:])
                sbT.append(s_t)
            # Transpose nfh row to a (128, kh) column pair via K=1 matmuls.
            p_nfh = ps.tile([128, 2], dt, tag="nfhT")
            for kh in range(2):
                nc.tensor.matmul(
                    p_nfh[:, kh : kh + 1],
                    nfh_row[:, kh * 128 : (kh + 1) * 128],
                    one_cin,
                    start=True,
                    stop=True,
                )
            sb_nfh_col = sb.tile([128, 2], dt)
            nc.vector.tensor_copy(out=sb_nfh_col[:], in_=p_nfh[:])

            # s_hid row (1, 14) and node_vals (14, 256), contracting h in 2 halves.
            ps_s = ps.tile([1, BN], dt, tag="s")
            ps_nv = ps.tile([BN, C], dt, tag="nv")
            for kh in range(2):
                nc.tensor.matmul(
                    ps_s[:],
                    sb_nfh_col[:, kh : kh + 1],
                    sbT[kh][:],
                    start=(kh == 0),
                    stop=(kh == 1),
                )
                nc.tensor.matmul(
                    ps_nv[:],
                    sbT[kh][:],
                    sb_w[:, kh * C : (kh + 1) * C],
                    start=(kh == 0),
                    stop=(kh == 1),
                )
            sb_nv = sb.tile([BN, C], dt)
            nc.vector.tensor_copy(out=sb_nv[:], in_=ps_nv[:])

            # Softmax over the 7 nodes (free dim), separately per local batch.
            e_row = sb.tile([1, BN], dt)
            denom = sb.tile([1, BL], dt)
            recip = sb.tile([1, BL], dt)
            a_row = sb.tile([1, BN], dt)
            for b in range(BL):
                nc.scalar.activation(
                    e_row[:, b * NODES : (b + 1) * NODES],
                    ps_s[:, b * NODES : (b + 1) * NODES],
                    mybir.ActivationFunctionType.Exp,
                    accum_out=denom[:, b : b + 1],
                )
            nc.vector.reciprocal(recip[:], denom[:])
            for b in range(BL):
                nc.vector.tensor_scalar_mul(
                    a_row[:, b * NODES : (b + 1) * NODES],
                    e_row[:, b * NODES : (b + 1) * NODES],
                    recip[:, b : b + 1],
                )

            # Transpose attn row to a column via K=1 matmul: ps_a[(b,n), 0] = a[b, n],
            # then expand into a block-diagonal (14, BL) matrix so one matmul per
            # c-half computes v for both local batches.
            ps_a = ps.tile([BN, 1], dt, tag="a")
            nc.tensor.matmul(ps_a[:], a_row[:], ones11[:], start=True, stop=True)
            sb_a = sb.tile([BN, 1], dt)
            nc.vector.tensor_copy(out=sb_a[:], in_=ps_a[:])
            rhs_a = sb.tile([BN, BL], dt)
            nc.vector.tensor_scalar_mul(rhs_a[:], blkmask[:], sb_a[:])

            # v[c, (ch, b)] = sum_n a[b, n] * nv[(b,n), c]; relu; broadcast; store.
            ps_v = ps.tile([128, 2 * BL], dt, tag="v")
            for ch in range(2):
                nc.tensor.matmul(
                    ps_v[:, ch * BL : (ch + 1) * BL],
                    sb_nv[:, ch * 128 : (ch + 1) * 128],
                    rhs_a[:],
                    start=True,
                    stop=True,
                )
            sb_v = sb.tile([128, 2 * BL], dt)
            nc.scalar.activation(sb_v[:], ps_v[:], mybir.ActivationFunctionType.Relu)
            # One broadcast tile + one DMA per local batch: out[b] is (256, P)
            # contiguous in DRAM, viewed as [p, ch, pix] with c = ch*128 + p.
            # Two DMAs let batch 1's broadcast fills overlap batch 0's store;
            # _fix_tail_drain spreads the resulting queue waits over spare
            # zero-wait tail drains.
            for b in range(BL):
                bc = sb.tile([128, 2, P], dt, tag=f"bc{b}")
                for ch in range(2):
                    j = ch * BL + b
                    nc.vector.tensor_copy(
                        out=bc[:, ch, :], in_=sb_v[:, j : j + 1].to_broadcast([128, P])
                    )
                nc.scalar.dma_start(
                    out=out_d[b].rearrange("(ch p) pix -> p ch pix", p=128),
                    in_=bc[:],
                )
    _fix_tail_drain(nc)
    return nc


def _fix_tail_drain(nc):
    """Walrus in this toolchain accepts very few sync waits per instruction, and
    Tile's kernel-tail drain waits on every semaphore. In this kernel the whole
    dataflow is one chain ending in the single output DMA: every other sem tick
    (input-DMA queue, PE, DVE, ACT) is strictly upstream of the output-DMA
    trigger, so waiting on the output queue's completion sem alone is
    sufficient. Strip the drain down to that one wait."""
    import bass_rust

    out_sem = None
    for ins in nc.inst_map.values():
        if type(ins).__name__ == "InstDMACopy" and "out_set" in str(ins):
            si = ins.sync_info
            if si is not None and len(si.on_update) > 0:
                out_sem = si.on_update[0].ant_name
    assert out_sem is not None, "output DMA completion sem not found"
    for ins in nc.inst_map.values():
        si = ins.sync_info
        if type(ins).__name__ == "InstDrain" and si is not None and len(si.on_wait) > 1:
            keep = [w for w in si.on_wait if w.ant_name == out_sem]
            assert len(keep) == 1, (out_sem, [w.ant_name for w in si.on_wait])
            ins.sync_info = bass_rust.SyncInfo(
                on_wait=keep, on_update=list(si.on_update)
            )


def _get_nc():
    if "nc" not in _cache:
        _cache["nc"] = _build_nc()
    return _cache["nc"]


def _pack_cin(x_shard, nfh, w):
    """Pack one core's inputs into the (128, CIN_COLS) tensor."""
    cin = np.zeros((128, CIN_COLS), dtype=np.float32)
    # w: [kh*128+k, c] -> [k, kh*256+c]
    cin[:, 0:C] = w[0:128, :]
    cin[:, C : 2 * C] = w[128:256, :]
    cin[XROW : XROW + BN, COL_ID : COL_ID + BN] = np.eye(BN, dtype=np.float32)
    for b in range(BL):
        cin[b * NODES : (b + 1) * NODES, COL_BM + b] = 1.0
    cin[0, COL_XN : COL_XN + HID] = nfh[:, 0]
    cin[XROW : XROW + BN, COL_XN : COL_XN + HID] = x_shard.reshape(BN, HID)
    cin[0, COL_ONE] = 1.0
    return cin


def _make_in_maps(input, node_fea_for_hidden, weight):
    x_full = np.asarray(input, dtype=np.float32)[0]  # (B, N, HID)
    nfh = np.asarray(node_fea_for_hidden, dtype=np.float32)
    w = np.asarray(weight, dtype=np.float32)
    return [
        {"cin": _pack_cin(x_full[i * BL : (i + 1) * BL], nfh, w)}
        for i in range(NCORES)
    ]


def _run(in_maps, trace=False, **kwargs):
    nc = _get_nc()
    return run_bass_kernel_spmd(nc, in_maps, list(range(NCORES)), trace=trace, **kwargs)


def kernel(input, res_feature, node_fea_for_res, node_fea_for_hidden, weight):
    in_maps = _make_in_maps(input, node_fea_for_hidden, weight)
    res = _run(in_maps)
    shards = [res.results[i]["out"] for i in range(NCORES)]  # each (BL, C, P)
    full = np.concatenate(shards, axis=0)  # (B, C, P)
    return full.reshape(B, C, H, W).astype(np.float32, copy=False)



# revision 10
# speedup vs baseline: 1.0745x; 1.0745x over previous
"""Trainium2 Bass kernel for nn_Graph_to_Featuremaps_savemem.

Reference computation:
    scores[b,p,n] = s_res[b,p] + s_hid[b,n];  attn = softmax_n(scores)
    out[b,c,p]    = relu(sum_n attn[b,p,n] * (x[b,n,:] @ W)[c])

Key simplification: softmax over n is shift-invariant, so the per-pixel
s_res[b,p] term (the only use of res_feature / node_fea_for_res) cancels:
    attn[b,p,n] = softmax_n(s_hid[b,n])   (independent of p)
    out[b,c,p]  = relu(sum_n a[b,n] * nv[b,n,c])  broadcast over all pixels.

So the kernel is a tiny softmax-weighted matmul per batch followed by a
151 MB broadcast-write of the (B, C) result over H*W pixels. Sharding:
data-parallel over batch, 2 batches per core across 8 cores; the small
params (node_fea_for_hidden, weight) are replicated.

The structure targets the DMA-store roofline (~26 GB/s x 16 DMA engines
per core): the 18.9 MB/core output stream goes on the wire as early as
possible and everything else hides beneath it.

  - The output broadcast is NOT materialized in SBUF.  Per (batch, c-half)
    only one (128, CH) chunk is filled (CH = 2304 pixels); the store DMA's
    *source* access pattern revisits it with a stride-0 repeat dim, so the
    DMA replicates it across all 9216 pixels.  This removes the baseline's
    2x 9.4 MB DVE broadcast fills (23.8 us) from the critical path.
    CH is chosen so descriptors are 9.2 KB: at 4.6 KB the descriptor
    generator falls ~6% short of the 16-engine line rate and the last
    engine in the round-robin accumulates an 8 us straggle.
  - All DMAs ride the sync-engine queue (its trigger is ~2x faster than
    the scalar engine's, and queue FIFO order lets the tail drain wait on
    the final DMA's semaphore alone).
  - s_hid = x . nfh is a DVE multiply + free-dim reduce against a
    host-packed nfh replica -- no PE transposes anywhere.
  - softmax normalization is deferred: y = x^T (blockmask * exp(s)) and
    v = W^T y use unnormalized weights; 1/denom and the ReLU are fused
    into the chunk fills (DVE tensor_scalar mult+max for the low c-half,
    scalar-engine activation Relu-with-scale for the high c-half, running
    concurrently).  v and 1/denom are funneled through GPSIMD copies so
    every fill carries a single sync wait (HW limit).
  - matmuls run in bf16 (O(1) gaussian data; tolerance 2e-2, measured
    error ~3e-3).
"""

import numpy as np

import concourse.bass as bass
import concourse.mybir as mybir
import concourse.tile as tile
from concourse.bass_utils import run_bass_kernel_spmd

B, NODES, HID, C, H, W = 16, 7, 256, 256, 96, 96
P = H * W                # 9216 pixels
NCORES = 8
BL = B // NCORES         # 2 local batches per core
BN = BL * NODES          # 14 (b,n) rows
CH = 2304                # materialized chunk width (pixels)
NREP = P // CH           # stride-0 repeats in the store DMA

# cin_a (small, loaded first; only rows 32:46 are transferred):
#   cols 0:256 x[(b n), h]; 256:512 nfh replicated per row; 512:514 blockmask
XROW = 32                # base partition for the 14 (b,n) rows (PE: 0/32/64)
COL_X = 0
COL_NFH = 256
COL_BM = 512
CINA_COLS = 514
# cin_b: W packed [k, kh*256 + c] (k = h % 128, kh = h // 128)
CINB_COLS = 2 * C

_cache: dict = {}


def _rep_ap(ap, dims):
    """Return a copy of `ap` with its non-partition dims replaced by `dims`
    (list of [stride, count]); used to build stride-0 broadcast patterns."""
    a = ap.copy()
    a.ap = mybir.VecI64Pair([list(a.ap[0])] + [list(d) for d in dims])
    return a


def _build_nc():
    nc = bass.Bass()
    f32 = mybir.dt.float32
    bf16 = mybir.dt.bfloat16
    cina_d = nc.declare_dram_parameter("cina", [128, CINA_COLS], f32, isOutput=False)
    cinb_d = nc.declare_dram_parameter("cinb", [128, CINB_COLS], f32, isOutput=False)
    out_d = nc.declare_dram_parameter("out", [BL, C, P], f32, isOutput=True)

    with tile.TileContext(nc) as tc:
        with (
            tc.tile_pool(name="sb", bufs=1) as sb,
            tc.tile_pool(name="ps", bufs=1, space=bass.MemorySpace.PSUM) as ps,
        ):
            cina = sb.tile([128, CINA_COLS], f32)
            cinb = sb.tile([128, CINB_COLS], f32)
            # Small x/nfh/mask part first, weight second, both on the sync
            # queue (fast trigger).
            nc.sync.dma_start(
                out=cina[XROW : XROW + BN, :], in_=cina_d[XROW : XROW + BN, :]
            )
            nc.sync.dma_start(out=cinb[:], in_=cinb_d[:])

            x_sl = cina[XROW : XROW + BN, COL_X : COL_X + HID]
            nfh_sl = cina[XROW : XROW + BN, COL_NFH : COL_NFH + HID]
            bm_sl = cina[XROW : XROW + BN, COL_BM : COL_BM + BL]

            # DVE-produced matmul operands (single-producer rule for PE).
            ones_col = sb.tile([128, 1], bf16)
            nc.vector.memset(ones_col[:], 1.0)
            ones_row = sb.tile([1, 128], bf16)
            nc.vector.memset(ones_row[:], 1.0)

            # s[(b n)] = sum_h x * nfh  (multiply + free-dim reduce).
            tt_scratch = sb.tile([128, HID], f32)
            s_col = sb.tile([128, 1], f32)
            nc.vector.tensor_tensor(
                out=tt_scratch[XROW : XROW + BN, :],
                in0=x_sl,
                in1=nfh_sl,
                op=mybir.AluOpType.mult,
            )
            nc.vector.tensor_reduce(
                out=s_col[XROW : XROW + BN, :],
                in_=tt_scratch[XROW : XROW + BN, :],
                axis=mybir.AxisListType.X,
                op=mybir.AluOpType.add,
            )
            sb_x = sb.tile([128, HID], bf16)
            nc.vector.tensor_copy(out=sb_x[XROW : XROW + BN, :], in_=x_sl)
            sb_w = sb.tile([128, CINB_COLS], bf16)
            nc.vector.tensor_copy(out=sb_w[:], in_=cinb[:])

            # e = exp(s) on the scalar engine (normalization deferred).
            e_col = sb.tile([128, 1], f32)
            nc.scalar.activation(
                e_col[XROW : XROW + BN, :],
                s_col[XROW : XROW + BN, :],
                mybir.ActivationFunctionType.Exp,
            )
            # rhs_e[(b n), b'] = blockmask * e  (unnormalized per-batch attn).
            rhs_e = sb.tile([128, BL], bf16)
            nc.vector.tensor_scalar(
                out=rhs_e[XROW : XROW + BN, :],
                in0=bm_sl,
                scalar1=e_col[XROW : XROW + BN, 0:1],
                scalar2=None,
                op0=mybir.AluOpType.mult,
            )

            # denom[b] = sum_n e ; y[h, b] = sum_n x * e  (contract over bn).
            ps_den = ps.tile([1, BL], f32, tag="den")
            nc.tensor.matmul(
                ps_den[:],
                ones_col[XROW : XROW + BN, :],
                rhs_e[XROW : XROW + BN, :],
                start=True,
                stop=True,
            )
            ps_y = ps.tile([128, 2 * BL], f32, tag="y")
            for kh in range(2):
                nc.tensor.matmul(
                    ps_y[:, kh * BL : (kh + 1) * BL],
                    sb_x[XROW : XROW + BN, kh * 128 : (kh + 1) * 128],
                    rhs_e[XROW : XROW + BN, :],
                    start=True,
                    stop=True,
                )
            recip = sb.tile([1, BL], bf16)
            with nc.allow_low_precision(reason="1/denom in bf16; tol 2e-2"):
                nc.vector.reciprocal(recip[:], ps_den[:])
            s_y = sb.tile([128, 2 * BL], bf16)
            nc.vector.tensor_copy(out=s_y[:], in_=ps_y[:])

            # v[c, b] = sum_h W[h, c] * y[h, b]   (c-half per group).
            ps_v = ps.tile([128, 2 * BL], f32, tag="v")
            for ch in range(2):
                for kh in range(2):
                    nc.tensor.matmul(
                        ps_v[:, ch * BL : (ch + 1) * BL],
                        sb_w[:, kh * C + ch * 128 : kh * C + (ch + 1) * 128],
                        s_y[:, kh * BL : (kh + 1) * BL],
                        start=(kh == 0),
                        stop=(kh == 1),
                    )

            # Broadcast 1/denom to all partitions with a K=1 matmul (GPSIMD
            # cannot read SBUF->SBUF partition-wise and DVE lanes cannot
            # cross partitions), then funnel v and 1/denom to SBUF on DVE so
            # every fill below needs at most one sync wait (HW limit).
            ps_r = ps.tile([128, BL], f32, tag="r")
            nc.tensor.matmul(ps_r[:], ones_row[:], recip[:], start=True, stop=True)
            s_v = sb.tile([128, 2 * BL], f32)
            nc.vector.tensor_copy(out=s_v[:], in_=ps_v[:])
            s_rr = sb.tile([128, BL], f32)
            nc.vector.tensor_copy(out=s_rr[:], in_=ps_r[:])

            # Normalize + ReLU + broadcast-fill one CH-wide chunk per
            # (batch, c-half); the store DMA replicates it over all pixels
            # via a stride-0 repeat dim in its source access pattern.
            # Low c-half fills on DVE, high c-half fills on the scalar
            # engine (activation Relu with per-partition scale) -- the two
            # engines fill concurrently.
            for b in range(BL):
                bc = sb.tile([128, 2 * CH], f32, tag=f"bc{b}")
                nc.vector.tensor_scalar(
                    out=bc[:, 0:CH],
                    in0=_rep_ap(s_v[:, b : b + 1], [[0, CH]]),
                    scalar1=s_rr[:, b : b + 1],
                    scalar2=0.0,
                    op0=mybir.AluOpType.mult,
                    op1=mybir.AluOpType.max,
                )
                nc.sync.dma_start(
                    out=_rep_ap(out_d[b][0:128, :], [[CH, NREP], [1, CH]]),
                    in_=_rep_ap(bc[:, 0:CH], [[0, NREP], [1, CH]]),
                )
                nc.scalar.activation(
                    bc[:, CH : 2 * CH],
                    _rep_ap(s_v[:, BL + b : BL + b + 1], [[0, CH]]),
                    mybir.ActivationFunctionType.Relu,
                    scale=s_rr[:, b : b + 1],
                )
                nc.sync.dma_start(
                    out=_rep_ap(out_d[b][128:256, :], [[CH, NREP], [1, CH]]),
                    in_=_rep_ap(bc[:, CH : 2 * CH], [[0, NREP], [1, CH]]),
                )
    _fix_tail_drain(nc)
    return nc


def _fix_tail_drain(nc):
    """Walrus accepts very few sync waits per instruction, and Tile's
    kernel-tail drain waits on every semaphore. The whole dataflow funnels
    into the four output DMAs, all FIFO on the sync queue, so waiting on
    the LAST one's completion sem alone is sufficient. Strip the drain
    down to that wait."""
    import bass_rust

    out_sem = None
    for ins in nc.inst_map.values():
        if type(ins).__name__ == "InstDMACopy" and "out_set" in str(ins):
            si = ins.sync_info
            if si is not None and len(si.on_update) > 0:
                out_sem = si.on_update[0].ant_name
    assert out_sem is not None, "output DMA completion sem not found"
    for ins in nc.inst_map.values():
        si = ins.sync_info
        if type(ins).__name__ == "InstDrain" and si is not None and len(si.on_wait) > 1:
            keep = [w for w in si.on_wait if w.ant_name == out_sem]
            assert len(keep) == 1, (out_sem, [w.ant_name for w in si.on_wait])
            ins.sync_info = bass_rust.SyncInfo(
                on_wait=keep, on_update=list(si.on_update)
            )


def _get_nc():
    if "nc" not in _cache:
        _cache["nc"] = _build_nc()
    return _cache["nc"]


def _pack_cina(x_shard, nfh):
    cina = np.zeros((128, CINA_COLS), dtype=np.float32)
    cina[XROW : XROW + BN, COL_X : COL_X + HID] = x_shard.reshape(BN, HID)
    cina[XROW : XROW + BN, COL_NFH : COL_NFH + HID] = nfh[:, 0][None, :]
    for b in range(BL):
        cina[XROW + b * NODES : XROW + (b + 1) * NODES, COL_BM + b] = 1.0
    return cina


def _pack_cinb(w):
    cinb = np.zeros((128, CINB_COLS), dtype=np.float32)
    cinb[:, 0:C] = w[0:128, :]
    cinb[:, C : 2 * C] = w[128:256, :]
    return cinb


def _make_in_maps(input, node_fea_for_hidden, weight):
    x_full = np.asarray(input, dtype=np.float32)[0]  # (B, N, HID)
    nfh = np.asarray(node_fea_for_hidden, dtype=np.float32)
    w = np.asarray(weight, dtype=np.float32)
    cinb = _pack_cinb(w)
    return [
        {"cina": _pack_cina(x_full[i * BL : (i + 1) * BL], nfh), "cinb": cinb}
        for i in range(NCORES)
    ]


def _run(in_maps, trace=False, **kwargs):
    nc = _get_nc()
    return run_bass_kernel_spmd(nc, in_maps, list(range(NCORES)), trace=trace, **kwargs)


def kernel(input, res_feature, node_fea_for_res, node_fea_for_hidden, weight):
    in_maps = _make_in_maps(input, node_fea_for_hidden, weight)
    res = _run(in_maps)
    shards = [res.results[i]["out"] for i in range(NCORES)]  # each (BL, C, P)
    full = np.concatenate(shards, axis=0)  # (B, C, P)
    return full.reshape(B, C, H, W).astype(np.float32, copy=False)


# revision 13
# speedup vs baseline: 1.1646x; 1.0839x over previous
"""Trainium2 Bass kernel for nn_Graph_to_Featuremaps_savemem.

Reference computation:
    scores[b,p,n] = s_res[b,p] + s_hid[b,n];  attn = softmax_n(scores)
    out[b,c,p]    = relu(sum_n attn[b,p,n] * (x[b,n,:] @ W)[c])

Key simplification: softmax over n is shift-invariant, so the per-pixel
s_res[b,p] term (the only use of res_feature / node_fea_for_res) cancels:
    attn[b,p,n] = softmax_n(s_hid[b,n])   (independent of p)
    out[b,c,p]  = relu(sum_n a[b,n] * nv[b,n,c])  broadcast over all pixels.

So the kernel is a tiny softmax-weighted matmul per batch followed by a
151 MB broadcast-write of the (B, C) result over H*W pixels. Sharding:
data-parallel over batch, 2 batches per core across 8 cores; the small
params (node_fea_for_hidden, weight) are replicated.

The structure targets the DMA-store roofline (~26 GB/s x 16 DMA engines
per core): the 18.9 MB/core output stream goes on the wire as early as
possible and everything else hides beneath it.

  - The output broadcast is NOT materialized in SBUF.  Per (batch, c-half)
    only one (128, CH) chunk is filled (CH = 2304 pixels); the store DMA's
    *source* access pattern revisits it with a stride-0 repeat dim, so the
    DMA replicates it across all 9216 pixels.  This removes the baseline's
    2x 9.4 MB DVE broadcast fills (23.8 us) from the critical path.
    CH is chosen so descriptors are 9.2 KB: at 4.6 KB the descriptor
    generator falls ~6% short of the 16-engine line rate and the last
    engine in the round-robin accumulates an 8 us straggle.
  - All DMAs ride the sync-engine queue (its trigger is ~2x faster than
    the scalar engine's, and queue FIFO order lets the tail drain wait on
    the final DMA's semaphore alone).
  - s_hid = x . nfh is a DVE multiply + free-dim reduce against a
    host-packed nfh replica -- no PE transposes anywhere.
  - softmax normalization is deferred: y = x^T (blockmask * exp(s)) and
    v = W^T y use unnormalized weights; 1/denom and the ReLU are fused
    into the chunk fills (DVE tensor_scalar mult+max for the low c-half,
    scalar-engine activation Relu-with-scale for the high c-half, running
    concurrently).  v and 1/denom are funneled through GPSIMD copies so
    every fill carries a single sync wait (HW limit).
  - matmuls run in bf16 (O(1) gaussian data; tolerance 2e-2, measured
    error ~3e-3).
"""

import numpy as np

import concourse.bass as bass
import concourse.mybir as mybir
import concourse.tile as tile
from concourse.bass_utils import run_bass_kernel_spmd

B, NODES, HID, C, H, W = 16, 7, 256, 256, 96, 96
P = H * W                # 9216 pixels
NCORES = 8
BL = B // NCORES         # 2 local batches per core
BN = BL * NODES          # 14 (b,n) rows
CH = 4608                # materialized chunk width (pixels)
NREP = P // CH           # stride-0 repeats in the store DMA

# cin_a (small, loaded first; only rows 32:46 are transferred):
#   cols 0:256 x[(b n), h]; 256:512 nfh replicated per row; 512:514 blockmask
XROW = 32                # base partition for the 14 (b,n) rows (PE: 0/32/64)
COL_X = 0
COL_NFH = 256
COL_BM = 512
CINA_COLS = 514
# cin_b: W packed [k, kh*256 + c] (k = h % 128, kh = h // 128)
CINB_COLS = 2 * C

_cache: dict = {}


def _rep_ap(ap, dims):
    """Return a copy of `ap` with its non-partition dims replaced by `dims`
    (list of [stride, count]); used to build stride-0 broadcast patterns."""
    a = ap.copy()
    a.ap = mybir.VecI64Pair([list(a.ap[0])] + [list(d) for d in dims])
    return a


def _build_nc():
    nc = bass.Bass()
    f32 = mybir.dt.float32
    bf16 = mybir.dt.bfloat16
    cina_d = nc.declare_dram_parameter("cina", [128, CINA_COLS], f32, isOutput=False)
    cinb_d = nc.declare_dram_parameter("cinb", [128, CINB_COLS], f32, isOutput=False)
    out_d = nc.declare_dram_parameter("out", [BL, C, P], f32, isOutput=True)

    with tile.TileContext(nc) as tc:
        with (
            tc.tile_pool(name="sb", bufs=1) as sb,
            tc.tile_pool(name="ps", bufs=1, space=bass.MemorySpace.PSUM) as ps,
        ):
            cina = sb.tile([128, CINA_COLS], f32)
            cinb = sb.tile([128, CINB_COLS], f32)
            # Small x/nfh/mask part first, weight second, both on the sync
            # queue (fast trigger).
            nc.sync.dma_start(
                out=cina[XROW : XROW + BN, :], in_=cina_d[XROW : XROW + BN, :]
            )
            nc.sync.dma_start(out=cinb[:], in_=cinb_d[:])

            x_sl = cina[XROW : XROW + BN, COL_X : COL_X + HID]
            nfh_sl = cina[XROW : XROW + BN, COL_NFH : COL_NFH + HID]
            bm_sl = cina[XROW : XROW + BN, COL_BM : COL_BM + BL]

            # DVE-produced matmul operands (single-producer rule for PE).
            ones_col = sb.tile([128, 1], bf16)
            nc.vector.memset(ones_col[:], 1.0)
            ones_row = sb.tile([1, 128], bf16)
            nc.vector.memset(ones_row[:], 1.0)

            # s[(b n)] = sum_h x * nfh  (multiply + free-dim reduce).
            tt_scratch = sb.tile([128, HID], f32)
            s_col = sb.tile([128, 1], f32)
            nc.vector.tensor_tensor(
                out=tt_scratch[XROW : XROW + BN, :],
                in0=x_sl,
                in1=nfh_sl,
                op=mybir.AluOpType.mult,
            )
            nc.vector.tensor_reduce(
                out=s_col[XROW : XROW + BN, :],
                in_=tt_scratch[XROW : XROW + BN, :],
                axis=mybir.AxisListType.X,
                op=mybir.AluOpType.add,
            )
            sb_x = sb.tile([128, HID], bf16)
            nc.vector.tensor_copy(out=sb_x[XROW : XROW + BN, :], in_=x_sl)

            # e = exp(s) on the scalar engine (normalization deferred).
            e_col = sb.tile([128, 1], f32)
            nc.scalar.activation(
                e_col[XROW : XROW + BN, :],
                s_col[XROW : XROW + BN, :],
                mybir.ActivationFunctionType.Exp,
            )
            # rhs_e[(b n), b'] = blockmask * e  (unnormalized per-batch attn).
            rhs_e = sb.tile([128, BL], bf16)
            nc.vector.tensor_scalar(
                out=rhs_e[XROW : XROW + BN, :],
                in0=bm_sl,
                scalar1=e_col[XROW : XROW + BN, 0:1],
                scalar2=None,
                op0=mybir.AluOpType.mult,
            )
            # Weight cast placed AFTER rhs_e in the DVE stream: it is 430 ns
            # of DVE time and must not delay the critical exp->rhs_e->y path
            # (the tile scheduler keeps per-engine program order here).
            sb_w = sb.tile([128, CINB_COLS], bf16)
            nc.vector.tensor_copy(out=sb_w[:], in_=cinb[:])

            # denom[b] = sum_n e ; y[h, b] = sum_n x * e  (contract over bn).
            ps_den = ps.tile([1, BL], f32, tag="den")
            nc.tensor.matmul(
                ps_den[:],
                ones_col[XROW : XROW + BN, :],
                rhs_e[XROW : XROW + BN, :],
                start=True,
                stop=True,
            )
            ps_y = ps.tile([128, 2 * BL], f32, tag="y")
            for kh in range(2):
                nc.tensor.matmul(
                    ps_y[:, kh * BL : (kh + 1) * BL],
                    sb_x[XROW : XROW + BN, kh * 128 : (kh + 1) * 128],
                    rhs_e[XROW : XROW + BN, :],
                    start=True,
                    stop=True,
                )
            recip = sb.tile([1, BL], bf16)
            with nc.allow_low_precision(reason="1/denom in bf16; tol 2e-2"):
                nc.vector.reciprocal(recip[:], ps_den[:])
            s_y = sb.tile([128, 2 * BL], bf16)
            nc.vector.tensor_copy(out=s_y[:], in_=ps_y[:])

            # v[c, b] = sum_h W[h, c] * y[h, b]   (c-half per group).
            ps_v = ps.tile([128, 2 * BL], f32, tag="v")
            for ch in range(2):
                for kh in range(2):
                    nc.tensor.matmul(
                        ps_v[:, ch * BL : (ch + 1) * BL],
                        sb_w[:, kh * C + ch * 128 : kh * C + (ch + 1) * 128],
                        s_y[:, kh * BL : (kh + 1) * BL],
                        start=(kh == 0),
                        stop=(kh == 1),
                    )

            # Broadcast 1/denom to all partitions with a K=1 matmul (GPSIMD
            # cannot read SBUF->SBUF partition-wise and DVE lanes cannot
            # cross partitions), then funnel v and 1/denom to SBUF on DVE so
            # every fill below needs at most one sync wait (HW limit).
            ps_r = ps.tile([128, BL], f32, tag="r")
            nc.tensor.matmul(ps_r[:], ones_row[:], recip[:], start=True, stop=True)
            s_v = sb.tile([128, 2 * BL], f32)
            nc.vector.tensor_copy(out=s_v[:], in_=ps_v[:])
            s_rr = sb.tile([128, BL], f32)
            nc.vector.tensor_copy(out=s_rr[:], in_=ps_r[:])

            # Normalize + ReLU + broadcast-fill one CH-wide chunk per
            # (batch, c-half); the store DMA replicates it over all pixels
            # via a stride-0 repeat dim in its source access pattern.
            # Low c-half fills on DVE, high c-half fills on the scalar
            # engine (activation Relu with per-partition scale) -- the two
            # engines fill concurrently.
            for b in range(BL):
                bc = sb.tile([128, 2 * CH], f32, tag=f"bc{b}")
                nc.vector.tensor_scalar(
                    out=bc[:, 0:CH],
                    in0=_rep_ap(s_v[:, b : b + 1], [[0, CH]]),
                    scalar1=s_rr[:, b : b + 1],
                    scalar2=0.0,
                    op0=mybir.AluOpType.mult,
                    op1=mybir.AluOpType.max,
                )
                nc.sync.dma_start(
                    out=_rep_ap(out_d[b][0:128, :], [[CH, NREP], [1, CH]]),
                    in_=_rep_ap(bc[:, 0:CH], [[0, NREP], [1, CH]]),
                )
                nc.scalar.activation(
                    bc[:, CH : 2 * CH],
                    _rep_ap(s_v[:, BL + b : BL + b + 1], [[0, CH]]),
                    mybir.ActivationFunctionType.Relu,
                    scale=s_rr[:, b : b + 1],
                )
                nc.sync.dma_start(
                    out=_rep_ap(out_d[b][128:256, :], [[CH, NREP], [1, CH]]),
                    in_=_rep_ap(bc[:, CH : 2 * CH], [[0, NREP], [1, CH]]),
                )
    _fix_tail_drain(nc)
    return nc


def _fix_tail_drain(nc):
    """Walrus accepts very few sync waits per instruction, and Tile's
    kernel-tail drain waits on every semaphore. The whole dataflow funnels
    into the four output DMAs, all FIFO on the sync queue, so waiting on
    the LAST one's completion sem alone is sufficient. Strip the drain
    down to that wait."""
    import bass_rust

    out_sem = None
    for ins in nc.inst_map.values():
        if type(ins).__name__ == "InstDMACopy" and "out_set" in str(ins):
            si = ins.sync_info
            if si is not None and len(si.on_update) > 0:
                out_sem = si.on_update[0].ant_name
    assert out_sem is not None, "output DMA completion sem not found"
    for ins in nc.inst_map.values():
        si = ins.sync_info
        if type(ins).__name__ == "InstDrain" and si is not None and len(si.on_wait) > 1:
            keep = [w for w in si.on_wait if w.ant_name == out_sem]
            assert len(keep) == 1, (out_sem, [w.ant_name for w in si.on_wait])
            ins.sync_info = bass_rust.SyncInfo(
                on_wait=keep, on_update=list(si.on_update)
            )


def _get_nc():
    if "nc" not in _cache:
        _cache["nc"] = _build_nc()
    return _cache["nc"]


def _pack_cina(x_shard, nfh):
    cina = np.zeros((128, CINA_COLS), dtype=np.float32)
    cina[XROW : XROW + BN, COL_X : COL_X + HID] = x_shard.reshape(BN, HID)
    cina[XROW : XROW + BN, COL_NFH : COL_NFH + HID] = nfh[:, 0][None, :]
    for b in range(BL):
        cina[XROW + b * NODES : XROW + (b + 1) * NODES, COL_BM + b] = 1.0
    return cina


def _pack_cinb(w):
    cinb = np.zeros((128, CINB_COLS), dtype=np.float32)
    cinb[:, 0:C] = w[0:128, :]
    cinb[:, C : 2 * C] = w[128:256, :]
    return cinb


def _make_in_maps(input, node_fea_for_hidden, weight):
    x_full = np.asarray(input, dtype=np.float32)[0]  # (B, N, HID)
    nfh = np.asarray(node_fea_for_hidden, dtype=np.float32)
    w = np.asarray(weight, dtype=np.float32)
    cinb = _pack_cinb(w)
    return [
        {"cina": _pack_cina(x_full[i * BL : (i + 1) * BL], nfh), "cinb": cinb}
        for i in range(NCORES)
    ]


def _run(in_maps, trace=False, **kwargs):
    nc = _get_nc()
    return run_bass_kernel_spmd(nc, in_maps, list(range(NCORES)), trace=trace, **kwargs)


def kernel(input, res_feature, node_fea_for_res, node_fea_for_hidden, weight):
    in_maps = _make_in_maps(input, node_fea_for_hidden, weight)
    res = _run(in_maps)
    shards = [res.results[i]["out"] for i in range(NCORES)]  # each (BL, C, P)
    full = np.concatenate(shards, axis=0)  # (B, C, P)
    return full.reshape(B, C, H, W).astype(np.float32, copy=False)
